# revision 6
# baseline (speedup 1.0000x reference)
"""Gated-relative-position-bias multi-head attention, 8-way tensor-parallel
over heads on Trainium2 (Bass/Tile).  v2 — PE-stall-free redesign.

Shapes: x (2, 2048, 1024), 16 heads x 64 head-dim, position_bias
(16, 2048, 2048), per-query sigmoid gates computed from x.

Sharding: core c owns heads (2c, 2c+1) = feature slice [128c, 128c+128).
Each core computes q/k/v for its heads, the gated-bias attention, and a
partial output projection (O_g @ Wo_g.T) written in fp16.  The host sums
the 8 partials and adds bo (+ the exact fold of bv through Wo: attention
rows sum to 1, so v-bias contributes bv @ Wo.T to every output row).

Key structure (all engines balanced, PE never waits):
  - scores are computed TRANSPOSED, sT[k, q] = kT.T @ qT (K=hd=64), with
    TWO query-halves run CONCURRENTLY via PE row tiling: the second
    matmul uses partition-swapped copies of k/q (kswap/qswap, built by
    SBUF->SBUF DMA) so it lands on PE rows 64-127 while the first uses
    rows 0-63.  This halves score-matmul wall time.
  - the gated position bias is added into the score PSUM by the PE via
    an identity matmul (psum += I.T @ pbg); pbg = pbT * gate_bcast is
    formed on the DVE in bf16 (2x mode).  gate_bcast is built by a K=1
    ones-matmul on the PE (no gpsimd anywhere in this kernel).
  - softmax needs no max-subtraction (scores are O(+-1) for this model
    family); denominators come free as an all-ones column of v_aug
    (row 64 of the AV matmul PSUM output).
  - exp runs on ACT as one [128,1024] pass per query-pair straight out
    of the 2-bank score PSUM tile.
  - normalization happens during the avs PSUM evacuation (DVE multiply
    by a PE-broadcast reciprocal); the output projection runs on the
    normalized OT and is evacuated by DVE into fp16.
"""

import sys

sys.path.insert(0, "/opt/trn_rl_repo")

import ml_dtypes
import numpy as np

import concourse.mybir as mybir
import concourse.tile as tile
from concourse import bacc
from concourse.bass_utils import run_bass_kernel_spmd

F32 = mybir.dt.float32
BF16 = mybir.dt.bfloat16
FP16 = mybir.dt.float16
AF = mybir.ActivationFunctionType
ALU = mybir.AluOpType

B, T, D, H, HD = 2, 2048, 1024, 16, 64
NCORES = 8
HPC = H // NCORES          # heads per core = 2
FPC = HPC * HD             # features per core = 128
BT = B * T                 # 4096
P = 128
NKC = T // P               # key chunks per (h, b) = 16
NTH = BT // 512            # 512-col projection tiles = 8

# test.py hooks
TRACE = False
LAST_RESULT = None


def _build_program():
    nc = bacc.Bacc("TRN2", target_bir_lowering=False, debug=False,
                   num_devices=NCORES)

    xT = nc.dram_tensor("xT", [D, BT], BF16, kind="ExternalInput")
    xg = nc.dram_tensor("xg", [P, BT], BF16, kind="ExternalInput")
    wq = nc.dram_tensor("wq", [D, FPC], BF16, kind="ExternalInput")
    wk = nc.dram_tensor("wk", [D, FPC], BF16, kind="ExternalInput")
    wv = nc.dram_tensor("wv", [D, FPC], BF16, kind="ExternalInput")
    bq = nc.dram_tensor("bq", [FPC], F32, kind="ExternalInput")
    bk = nc.dram_tensor("bk", [FPC], F32, kind="ExternalInput")
    wo = nc.dram_tensor("wo", [FPC, D], BF16, kind="ExternalInput")
    pbt = nc.dram_tensor("pbt", [HPC, T, T], BF16, kind="ExternalInput")
    wg2 = nc.dram_tensor("wg2", [P, 97], BF16, kind="ExternalInput")
    bg2 = nc.dram_tensor("bg2", [97], F32, kind="ExternalInput")
    gc2 = nc.dram_tensor("gc2", [97], F32, kind="ExternalInput")
    idb = nc.dram_tensor("idb", [P, P], BF16, kind="ExternalInput")
    out = nc.dram_tensor("out", [BT, D], FP16, kind="ExternalOutput")

    with tile.TileContext(nc) as tc, \
         tc.tile_pool(name="const", bufs=1) as const, \
         tc.tile_pool(name="big", bufs=1) as big, \
         tc.tile_pool(name="xt", bufs=2) as xt_pool, \
         tc.tile_pool(name="gtmp", bufs=2) as gtmp_pool, \
         tc.tile_pool(name="pb", bufs=3) as pb_pool, \
         tc.tile_pool(name="pbgp", bufs=2) as pbg_pool, \
         tc.tile_pool(name="exp", bufs=4) as ex_pool, \
         tc.tile_pool(name="dscp", bufs=4) as dsc_pool, \
         tc.tile_pool(name="rbrp", bufs=2) as rbr_pool, \
         tc.tile_pool(name="obp", bufs=3) as ob_pool, \
         tc.tile_pool(name="ps", bufs=2, space="PSUM") as ps:
        # ---------------- constants ----------------
        ones1 = const.tile([P, P], BF16, tag="ones")
        nc.vector.memset(ones1[:], 1.0)
        idb_t = const.tile([P, P], BF16, tag="idb")
        nc.sync.dma_start(idb_t[:], idb[:])
        w_ts = {}
        for name, dram in (("wq", wq), ("wk", wk), ("wv", wv)):
            w_t = const.tile([P, D // P, FPC], BF16, tag=name, name=name + "w")
            nc.sync.dma_start(w_t[:], dram.rearrange("(c p) f -> p c f", p=P))
            w_ts[name] = w_t
        b_ts = {}
        for name, dram in (("bq", bq), ("bk", bk)):
            b_t = const.tile([FPC, 1], F32, tag=name, name=name + "b")
            nc.sync.dma_start(b_t[:], dram.rearrange("(p o) -> p o", o=1))
            b_ts[name] = b_t
        wo_t = const.tile([FPC, D], BF16, tag="wo")
        nc.sync.dma_start(wo_t[:], wo[:])
        wg2_t = const.tile([P, 97], BF16, tag="wg2")
        nc.sync.dma_start(wg2_t[:], wg2[:])
        bg2_t = const.tile([97, 1], F32, tag="bg2")
        nc.sync.dma_start(bg2_t[:], bg2.rearrange("(p o) -> p o", o=1))
        gc_t = const.tile([97, 1], F32, tag="gc")
        nc.sync.dma_start(gc_t[:], gc2.rearrange("(p o) -> p o", o=1))

        qT = big.tile([FPC, BT], BF16, tag="qT")
        kT = big.tile([FPC, BT], BF16, tag="kT")
        qsw = big.tile([FPC, BT], BF16, tag="qsw")
        ksw = big.tile([FPC, BT], BF16, tag="ksw")
        vT = big.tile([FPC, BT], BF16, tag="vT")
        G = big.tile([97, BT], BF16, tag="G")
        G2h = [big.tile([1, BT], BF16, tag=f"G2h{h}", name=f"G2h{h}")
               for h in range(HPC)]
        gbc = {(h, b): big.tile([P, T], BF16, tag=f"gbc{h}{b}",
                                name=f"gbc{h}{b}")
               for h in range(HPC) for b in range(B)}
        va = {(h, b): big.tile([P, NKC * (HD + 1)], BF16,
                               tag=f"va{h}{b}", name=f"va{h}{b}")
              for h in range(HPC) for b in range(B)}
        OT = [big.tile([FPC, T], BF16, tag=f"OT{b}", name=f"OT{b}")
              for b in range(B)]

        # ones-columns of v_aug (data columns are overwritten by the
        # transpose evacuations below)
        for h in range(HPC):
            for b in range(B):
                vac = va[(h, b)][:, :].rearrange("p (k c) -> p k c", c=HD + 1)
                nc.vector.memset(vac[:, :, HD:HD + 1], 1.0)

        # ---------------- phase A: q/k/v projections + gate ----------------
        def emit_vaug(b):
            # vT[:, b] -> per-head transposed v chunks in va (PE row-paired
            # transposes, staged 8-at-a-time in a half PSUM bank)
            for half in range(2):
                stages = []
                for h in range(HPC):
                    hsl = slice(h * HD, (h + 1) * HD)
                    stage = ps.tile([P, 512], BF16, tag="av", bufs=4,
                                    name=f"vstg{b}{half}{h}")
                    for j in range(8):
                        kc = half * 8 + j
                        nc.tensor.transpose(
                            stage[:, j * HD:(j + 1) * HD],
                            vT[hsl, b * T + kc * P: b * T + (kc + 1) * P],
                            idb_t[hsl, hsl])
                    stages.append(stage)
                for h in range(HPC):
                    dst = va[(h, b)][:, half * 8 * (HD + 1):
                                     (half * 8 + 8) * (HD + 1)]
                    dst = dst.rearrange("p (k c) -> p k c", c=HD + 1)
                    src = stages[h][:, :].rearrange("p (k c) -> p k c", c=HD)
                    nc.vector.tensor_copy(dst[:, :, 0:HD], src[:])

        def emit_swaps(b):
            bsl = slice(b * T, (b + 1) * T)
            for src, dstt in ((kT, ksw), (qT, qsw)):
                nc.sync.dma_start(dstt[0:HD, bsl], src[HD:FPC, bsl])
                nc.sync.dma_start(dstt[HD:FPC, bsl], src[0:HD, bsl])

        for th in range(NTH):
            b = th // (NTH // B)
            sl = slice(th * 512, (th + 1) * 512)
            qsl = slice((th % 4) * 512, (th % 4) * 512 + 512)
            xts = []
            for d in range(D // P):
                xt_t = xt_pool.tile([P, 512], BF16, tag=f"xt{d}",
                                    name=f"xt{d}_{th}")
                nc.sync.dma_start(xt_t[:], xT[d * P:(d + 1) * P, sl])
                xts.append(xt_t)
            xg_t = xt_pool.tile([P, 512], BF16, tag="xg", name=f"xg_{th}")
            nc.sync.dma_start(xg_t[:], xg[:, sl])

            # gate: projection, sigmoid, per-head combine, PE broadcast
            psg = ps.tile([97, 512], F32, tag="sc", name=f"psg{th}")
            nc.tensor.matmul(psg[:], wg2_t[:], xg_t[:], start=True, stop=True)
            nc.scalar.activation(G[:, sl], psg[:], AF.Sigmoid, bias=bg2_t[:])
            gt1 = gtmp_pool.tile([97, 512], BF16, tag="gt1", name=f"gt1_{th}")
            for h in range(HPC):
                r = 32 * h
                nc.vector.tensor_scalar(
                    out=gt1[r:r + 1, :], in0=G[64 + r:65 + r, sl],
                    scalar1=gc_t[64 + r:65 + r, :], scalar2=-1.0,
                    op0=ALU.mult, op1=ALU.add)
                nc.vector.tensor_tensor(
                    out=G2h[h][0:1, sl], in0=G[r:r + 1, sl],
                    in1=gt1[r:r + 1, :], op=ALU.mult)
                nc.vector.tensor_scalar(
                    out=G2h[h][0:1, sl], in0=G2h[h][0:1, sl],
                    scalar1=2.0, scalar2=None, op0=ALU.add)
                psb = ps.tile([P, 512], F32, tag="sc", name=f"psb{th}{h}")
                nc.tensor.matmul(psb[:], ones1[0:1, :], G2h[h][0:1, sl],
                                 start=True, stop=True)
                if h == 0:
                    nc.scalar.copy(gbc[(h, b)][:, qsl], psb[:])
                else:
                    nc.vector.tensor_copy(gbc[(h, b)][:, qsl], psb[:])

            # q/k/v projections (weights-stationary, 8 accum steps)
            for wname, bname, dst in (("wq", "bq", qT), ("wk", "bk", kT),
                                      ("wv", None, vT)):
                psq = ps.tile([FPC, 512], F32, tag="sc",
                              name=f"ps{wname}{th}")
                for d in range(D // P):
                    nc.tensor.matmul(psq[:], w_ts[wname][:, d, :], xts[d][:],
                                     start=(d == 0), stop=(d == D // P - 1))
                if bname is not None:
                    nc.vector.tensor_scalar(
                        out=dst[:, sl], in0=psq[:], scalar1=b_ts[bname][:],
                        scalar2=None, op0=ALU.add)
                else:
                    nc.vector.tensor_copy(dst[:, sl], psq[:])

            if th % 4 == 3:          # batch b fully projected
                emit_swaps(b)
                emit_vaug(b)

        # ---------------- phase B: attention + output projection ----------
        blocks = [(0, 0), (1, 0), (0, 1), (1, 1)]
        pending_po = []              # (b, tt) out-proj chunks to interleave

        def emit_po(b, tt):
            po = ps.tile([P, 1024], F32, tag="sc", name=f"po{b}{tt}")
            lo = OT[b][:, tt * P:(tt + 1) * P]
            nc.tensor.matmul(po[:, 0:512], lo, wo_t[:, 0:512],
                             start=True, stop=True)
            nc.tensor.matmul(po[:, 512:1024], lo, wo_t[:, 512:1024],
                             start=True, stop=True)
            ob = ob_pool.tile([P, 1024], FP16, tag="ob", name=f"ob{b}{tt}")
            nc.vector.tensor_copy(ob[:], po[:])
            nc.sync.dma_start(out[b * T + tt * P: b * T + (tt + 1) * P, :],
                              ob[:])

        for h, b in blocks:
            hsl = slice(h * HD, (h + 1) * HD)
            va_t = va[(h, b)]
            gbc_t = gbc[(h, b)]
            kA = (kT if h == 0 else ksw)
            kB = (ksw if h == 0 else kT)
            qA = (qT if h == 0 else qsw)
            qB = (qsw if h == 0 else qT)
            avs = [ps.tile([HD + 1, 512], F32, tag="av", bufs=4,
                           name=f"avs{h}{b}{qq}") for qq in range(4)]
            for kc in range(NKC):
                if pending_po:
                    emit_po(*pending_po.pop(0))
                pbt_t = pb_pool.tile([P, T], BF16, tag="pb",
                                     name=f"pbt{h}{b}{kc}")
                nc.sync.dma_start(pbt_t[:], pbt[h, kc * P:(kc + 1) * P, :])
                pbg = pbg_pool.tile([P, T], BF16, tag="pbg",
                                    name=f"pbg{h}{b}{kc}")
                nc.vector.tensor_tensor(out=pbg[:], in0=pbt_t[:],
                                        in1=gbc_t[:], op=ALU.mult)
                ksl = slice(b * T + kc * P, b * T + (kc + 1) * P)
                for pi in range(2):
                    q0 = b * T + pi * 1024
                    sc2 = ps.tile([P, 1024], F32, tag="sc",
                                  name=f"sc{h}{b}{kc}{pi}")
                    nc.tensor.matmul(sc2[:, 0:512], kA[0:HD, ksl],
                                     qA[0:HD, q0:q0 + 512],
                                     start=True, stop=False)
                    nc.tensor.matmul(sc2[:, 512:1024], kB[HD:FPC, ksl],
                                     qB[HD:FPC, q0 + 512:q0 + 1024],
                                     start=True, stop=False)
                    nc.tensor.matmul(sc2[:, 0:512], idb_t[:],
                                     pbg[:, pi * 1024:pi * 1024 + 512],
                                     start=False, stop=True)
                    nc.tensor.matmul(sc2[:, 512:1024], idb_t[:],
                                     pbg[:, pi * 1024 + 512:(pi + 1) * 1024],
                                     start=False, stop=True)
                    ex = ex_pool.tile([P, 1024], BF16, tag="ex",
                                      name=f"ex{h}{b}{kc}{pi}")
                    nc.scalar.activation(ex[:], sc2[:], AF.Exp)
                    vak = va_t[:, kc * (HD + 1):(kc + 1) * (HD + 1)]
                    nc.tensor.matmul(avs[2 * pi][:], vak, ex[:, 0:512],
                                     start=(kc == 0), stop=(kc == NKC - 1))
                    nc.tensor.matmul(avs[2 * pi + 1][:], vak,
                                     ex[:, 512:1024],
                                     start=(kc == 0), stop=(kc == NKC - 1))
            # normalization: reciprocal of the free denominators (row HD),
            # PE-broadcast, multiply during avs evacuation into OT
            for qq in range(4):
                dsc = dsc_pool.tile([HD + 1, 512], BF16, tag="dsc",
                                    name=f"dsc{h}{b}{qq}")
                nc.scalar.copy(dsc[HD:HD + 1, :], avs[qq][HD:HD + 1, :])
                rbp = ps.tile([HD, 512], F32, tag="sc",
                              name=f"rbp{h}{b}{qq}")
                nc.tensor.matmul(rbp[:], ones1[HD:HD + 1, 0:HD],
                                 dsc[HD:HD + 1, :], start=True, stop=True)
                rbr = rbr_pool.tile([HD, 512], F32, tag="rbr",
                                    name=f"rbr{h}{b}{qq}")
                nc.vector.reciprocal_approx_fast(rbr[:], rbp[:])
                nc.vector.tensor_tensor(
                    out=OT[b][hsl, qq * 512:(qq + 1) * 512],
                    in0=avs[qq][0:HD, :], in1=rbr[:], op=ALU.mult)
            if h == 1:
                pending_po.extend((b, tt) for tt in range(T // P))

        while pending_po:
            emit_po(*pending_po.pop(0))

    nc.compile()
    return nc


_PROGRAM = None


def _get_program():
    global _PROGRAM
    if _PROGRAM is None:
        _PROGRAM = _build_program()
    return _PROGRAM


def kernel(x, position_bias, Wq, bq, Wk, bk, Wv, bv, Wo, bo, Wg, bg,
           gru_const):
    global LAST_RESULT
    x = np.asarray(x, dtype=np.float32)
    position_bias = np.asarray(position_bias, dtype=np.float32)
    Wq = np.asarray(Wq, dtype=np.float32)
    Wk = np.asarray(Wk, dtype=np.float32)
    Wv = np.asarray(Wv, dtype=np.float32)
    Wo = np.asarray(Wo, dtype=np.float32)
    bq = np.asarray(bq, dtype=np.float32)
    bk = np.asarray(bk, dtype=np.float32)
    bv = np.asarray(bv, dtype=np.float32)
    bo = np.asarray(bo, dtype=np.float32)
    Wg = np.asarray(Wg, dtype=np.float32)
    bg = np.asarray(bg, dtype=np.float32)
    gru_const = np.asarray(gru_const, dtype=np.float32)

    scale = np.float32(1.0 / np.sqrt(np.float32(HD)))

    xT_np = np.ascontiguousarray(x.reshape(BT, D).T)           # [D, BT]
    idb_np = np.eye(P).astype(ml_dtypes.bfloat16)
    # the reshape-(2,4)-sum of the 8 gate features is linear -> fold into
    # the weights:  Wg2[g] = sum of Wg rows [4g, 4g+4)
    Wg2 = Wg.reshape(2, 4, HD).sum(1)                          # [2, HD]
    bg2v = bg.reshape(2, 4).sum(1)                             # [2]

    in_maps = []
    for c in range(NCORES):
        fsl = slice(c * FPC, (c + 1) * FPC)
        wg2_np = np.zeros((P, 97), dtype=np.float32)
        bg2_np = np.zeros((97,), dtype=np.float32)
        # rows 0/32 = gate-a for head0/head1; rows 64/96 = gate-b
        wg2_np[0:HD, 0] = Wg2[0]
        wg2_np[HD:P, 32] = Wg2[0]
        wg2_np[0:HD, 64] = Wg2[1]
        wg2_np[HD:P, 96] = Wg2[1]
        bg2_np[[0, 32]] = bg2v[0]
        bg2_np[[64, 96]] = bg2v[1]
        gc2_np = np.zeros((97,), dtype=np.float32)
        gc2_np[64] = gru_const[0, c * HPC, 0, 0]
        gc2_np[96] = gru_const[0, c * HPC + 1, 0, 0]
        in_maps.append({
            "xT": xT_np.astype(ml_dtypes.bfloat16),
            "xg": np.ascontiguousarray(xT_np[fsl, :]).astype(ml_dtypes.bfloat16),
            "wq": (np.ascontiguousarray(Wq.T[:, fsl]) * scale).astype(ml_dtypes.bfloat16),
            "wk": np.ascontiguousarray(Wk.T[:, fsl]).astype(ml_dtypes.bfloat16),
            "wv": np.ascontiguousarray(Wv.T[:, fsl]).astype(ml_dtypes.bfloat16),
            "bq": np.ascontiguousarray(bq[fsl]) * scale,
            "bk": np.ascontiguousarray(bk[fsl]),
            "wo": np.ascontiguousarray(Wo[:, fsl].T).astype(ml_dtypes.bfloat16),
            "pbt": np.ascontiguousarray(
                position_bias[c * HPC:(c + 1) * HPC].transpose(0, 2, 1)
            ).astype(ml_dtypes.bfloat16),
            "wg2": wg2_np.astype(ml_dtypes.bfloat16),
            "bg2": bg2_np,
            "gc2": gc2_np,
            "idb": idb_np,
        })

    nc = _get_program()
    res = run_bass_kernel_spmd(nc, in_maps, core_ids=list(range(NCORES)),
                               trace=TRACE)
    LAST_RESULT = res
    acc = res.results[0]["out"].astype(np.float32).copy()
    for c in range(1, NCORES):
        acc += res.results[c]["out"].astype(np.float32)
    # v-bias folds exactly through the projection (attn rows sum to 1)
    acc += bo[None, :] + (bv @ Wo.T)[None, :]
    return acc.reshape(B, T, D)


# revision 8
# speedup vs baseline: 1.0750x; 1.0750x over previous
"""Gated-relative-position-bias multi-head attention, 8-way tensor-parallel
over heads on Trainium2 (Bass/Tile).  v2 — PE-stall-free redesign.

Shapes: x (2, 2048, 1024), 16 heads x 64 head-dim, position_bias
(16, 2048, 2048), per-query sigmoid gates computed from x.

Sharding: core c owns heads (2c, 2c+1) = feature slice [128c, 128c+128).
Each core computes q/k/v for its heads, the gated-bias attention, and a
partial output projection (O_g @ Wo_g.T) written in fp16.  The host sums
the 8 partials and adds bo (+ the exact fold of bv through Wo: attention
rows sum to 1, so v-bias contributes bv @ Wo.T to every output row).

Key structure (all engines balanced, PE never waits):
  - scores are computed TRANSPOSED, sT[k, q] = kT.T @ qT (K=hd=64), with
    TWO query-halves run CONCURRENTLY via PE row tiling: the second
    matmul uses partition-swapped copies of k/q (kswap/qswap, built by
    SBUF->SBUF DMA) so it lands on PE rows 64-127 while the first uses
    rows 0-63.  This halves score-matmul wall time.
  - the gated position bias is added into the score PSUM by the PE via
    an identity matmul (psum += I.T @ pbg); pbg = pbT * gate_bcast is
    formed on the DVE in bf16 (2x mode).  gate_bcast is built by a K=1
    ones-matmul on the PE (no gpsimd anywhere in this kernel).
  - softmax needs no max-subtraction (scores are O(+-1) for this model
    family); denominators come free as an all-ones column of v_aug
    (row 64 of the AV matmul PSUM output).
  - exp runs on ACT as one [128,1024] pass per query-pair straight out
    of the 2-bank score PSUM tile.
  - normalization happens during the avs PSUM evacuation (DVE multiply
    by a PE-broadcast reciprocal); the output projection runs on the
    normalized OT and is evacuated by DVE into fp16.
"""

import sys

sys.path.insert(0, "/opt/trn_rl_repo")

import ml_dtypes
import numpy as np

import concourse.mybir as mybir
import concourse.tile as tile
from concourse import bacc
from concourse.bass_utils import run_bass_kernel_spmd

F32 = mybir.dt.float32
BF16 = mybir.dt.bfloat16
FP16 = mybir.dt.float16
AF = mybir.ActivationFunctionType
ALU = mybir.AluOpType

B, T, D, H, HD = 2, 2048, 1024, 16, 64
NCORES = 8
HPC = H // NCORES          # heads per core = 2
FPC = HPC * HD             # features per core = 128
BT = B * T                 # 4096
P = 128
NKC = T // P               # key chunks per (h, b) = 16
NTH = BT // 512            # 512-col projection tiles = 8

# test.py hooks
TRACE = False
LAST_RESULT = None


def _build_program():
    nc = bacc.Bacc("TRN2", target_bir_lowering=False, debug=False,
                   num_devices=NCORES)

    xT = nc.dram_tensor("xT", [D, BT], BF16, kind="ExternalInput")
    xg = nc.dram_tensor("xg", [P, BT], BF16, kind="ExternalInput")
    wq = nc.dram_tensor("wq", [D, FPC], BF16, kind="ExternalInput")
    wk = nc.dram_tensor("wk", [D, FPC], BF16, kind="ExternalInput")
    wv = nc.dram_tensor("wv", [D, FPC], BF16, kind="ExternalInput")
    bq = nc.dram_tensor("bq", [FPC], F32, kind="ExternalInput")
    bk = nc.dram_tensor("bk", [FPC], F32, kind="ExternalInput")
    wo = nc.dram_tensor("wo", [FPC, D], BF16, kind="ExternalInput")
    pbt = nc.dram_tensor("pbt", [HPC, T, T], BF16, kind="ExternalInput")
    wg2 = nc.dram_tensor("wg2", [P, 97], BF16, kind="ExternalInput")
    bg2 = nc.dram_tensor("bg2", [97], F32, kind="ExternalInput")
    gc2 = nc.dram_tensor("gc2", [97], F32, kind="ExternalInput")
    idb = nc.dram_tensor("idb", [P, P], BF16, kind="ExternalInput")
    out = nc.dram_tensor("out", [BT, D], FP16, kind="ExternalOutput")

    with tile.TileContext(nc) as tc, \
         tc.tile_pool(name="const", bufs=1) as const, \
         tc.tile_pool(name="big", bufs=1) as big, \
         tc.tile_pool(name="xt", bufs=2) as xt_pool, \
         tc.tile_pool(name="gtmp", bufs=2) as gtmp_pool, \
         tc.tile_pool(name="pb", bufs=3) as pb_pool, \
         tc.tile_pool(name="pbgp", bufs=2) as pbg_pool, \
         tc.tile_pool(name="exp", bufs=4) as ex_pool, \
         tc.tile_pool(name="dscp", bufs=4) as dsc_pool, \
         tc.tile_pool(name="rbrp", bufs=2) as rbr_pool, \
         tc.tile_pool(name="obp", bufs=3) as ob_pool, \
         tc.tile_pool(name="ps", bufs=2, space="PSUM") as ps:
        # ---------------- constants ----------------
        ones1 = const.tile([P, P], BF16, tag="ones")
        nc.vector.memset(ones1[:], 1.0)
        idb_t = const.tile([P, P], BF16, tag="idb")
        nc.sync.dma_start(idb_t[:], idb[:])
        w_ts = {}
        for name, dram in (("wq", wq), ("wk", wk), ("wv", wv)):
            w_t = const.tile([P, D // P, FPC], BF16, tag=name, name=name + "w")
            nc.sync.dma_start(w_t[:], dram.rearrange("(c p) f -> p c f", p=P))
            w_ts[name] = w_t
        b_ts = {}
        for name, dram in (("bq", bq), ("bk", bk)):
            b_t = const.tile([FPC, 1], F32, tag=name, name=name + "b")
            nc.sync.dma_start(b_t[:], dram.rearrange("(p o) -> p o", o=1))
            b_ts[name] = b_t
        wo_t = const.tile([FPC, D], BF16, tag="wo")
        nc.sync.dma_start(wo_t[:], wo[:])
        wg2_t = const.tile([P, 97], BF16, tag="wg2")
        nc.sync.dma_start(wg2_t[:], wg2[:])
        bg2_t = const.tile([97, 1], F32, tag="bg2")
        nc.sync.dma_start(bg2_t[:], bg2.rearrange("(p o) -> p o", o=1))
        gc_t = const.tile([97, 1], F32, tag="gc")
        nc.sync.dma_start(gc_t[:], gc2.rearrange("(p o) -> p o", o=1))

        qT = big.tile([FPC, BT], BF16, tag="qT")
        kT = big.tile([FPC, BT], BF16, tag="kT")
        qsw = big.tile([FPC, BT], BF16, tag="qsw")
        ksw = big.tile([FPC, BT], BF16, tag="ksw")
        vT = big.tile([FPC, BT], BF16, tag="vT")
        G = big.tile([97, BT], BF16, tag="G")
        G2h = [big.tile([1, BT], BF16, tag=f"G2h{h}", name=f"G2h{h}")
               for h in range(HPC)]
        gbc = {(h, b): big.tile([P, T], BF16, tag=f"gbc{h}{b}",
                                name=f"gbc{h}{b}")
               for h in range(HPC) for b in range(B)}
        va = {(h, b): big.tile([P, NKC * (HD + 1)], BF16,
                               tag=f"va{h}{b}", name=f"va{h}{b}")
              for h in range(HPC) for b in range(B)}
        OT = [big.tile([FPC, T], BF16, tag=f"OT{b}", name=f"OT{b}")
              for b in range(B)]

        # ones-columns of v_aug (data columns are overwritten by the
        # transpose evacuations below)
        for h in range(HPC):
            for b in range(B):
                vac = va[(h, b)][:, :].rearrange("p (k c) -> p k c", c=HD + 1)
                nc.vector.memset(vac[:, :, HD:HD + 1], 1.0)

        # ---------------- phase A: q/k/v projections + gate ----------------
        def emit_vaug(b):
            # vT[:, b] -> per-head transposed v chunks in va (PE row-paired
            # transposes, staged 8-at-a-time in a half PSUM bank)
            for half in range(2):
                stages = []
                for h in range(HPC):
                    hsl = slice(h * HD, (h + 1) * HD)
                    stage = ps.tile([P, 512], BF16, tag="av", bufs=4,
                                    name=f"vstg{b}{half}{h}")
                    for j in range(8):
                        kc = half * 8 + j
                        nc.tensor.transpose(
                            stage[:, j * HD:(j + 1) * HD],
                            vT[hsl, b * T + kc * P: b * T + (kc + 1) * P],
                            idb_t[hsl, hsl])
                    stages.append(stage)
                for h in range(HPC):
                    dst = va[(h, b)][:, half * 8 * (HD + 1):
                                     (half * 8 + 8) * (HD + 1)]
                    dst = dst.rearrange("p (k c) -> p k c", c=HD + 1)
                    src = stages[h][:, :].rearrange("p (k c) -> p k c", c=HD)
                    nc.vector.tensor_copy(dst[:, :, 0:HD], src[:])

        def emit_swaps(b):
            bsl = slice(b * T, (b + 1) * T)
            for src, dstt in ((kT, ksw), (qT, qsw)):
                nc.sync.dma_start(dstt[0:HD, bsl], src[HD:FPC, bsl])
                nc.sync.dma_start(dstt[HD:FPC, bsl], src[0:HD, bsl])

        for th in range(NTH):
            b = th // (NTH // B)
            sl = slice(th * 512, (th + 1) * 512)
            qsl = slice((th % 4) * 512, (th % 4) * 512 + 512)
            xt_t = xt_pool.tile([P, D // P, 512], BF16, tag="xt", bufs=3,
                                name=f"xt_{th}")
            nc.sync.dma_start(
                xt_t[:], xT.rearrange("(c p) t -> p c t", p=P)[:, :, sl])
            xts = [xt_t[:, d, :] for d in range(D // P)]
            xg_t = xt_pool.tile([P, 512], BF16, tag="xg", bufs=3,
                                name=f"xg_{th}")
            nc.sync.dma_start(xg_t[:], xg[:, sl])

            # gate: projection, sigmoid, per-head combine, PE broadcast
            psg = ps.tile([97, 512], F32, tag="sc", name=f"psg{th}")
            nc.tensor.matmul(psg[:], wg2_t[:], xg_t[:], start=True, stop=True)
            nc.scalar.activation(G[:, sl], psg[:], AF.Sigmoid, bias=bg2_t[:])
            gt1 = gtmp_pool.tile([97, 512], BF16, tag="gt1", name=f"gt1_{th}")
            for h in range(HPC):
                r = 32 * h
                nc.vector.tensor_scalar(
                    out=gt1[r:r + 1, :], in0=G[64 + r:65 + r, sl],
                    scalar1=gc_t[64 + r:65 + r, :], scalar2=-1.0,
                    op0=ALU.mult, op1=ALU.add)
                nc.vector.tensor_tensor(
                    out=G2h[h][0:1, sl], in0=G[r:r + 1, sl],
                    in1=gt1[r:r + 1, :], op=ALU.mult)
                nc.vector.tensor_scalar(
                    out=G2h[h][0:1, sl], in0=G2h[h][0:1, sl],
                    scalar1=2.0, scalar2=None, op0=ALU.add)
                psb = ps.tile([P, 512], F32, tag="sc", name=f"psb{th}{h}")
                nc.tensor.matmul(psb[:], ones1[0:1, :], G2h[h][0:1, sl],
                                 start=True, stop=True)
                if h == 0:
                    nc.scalar.copy(gbc[(h, b)][:, qsl], psb[:])
                else:
                    nc.vector.tensor_copy(gbc[(h, b)][:, qsl], psb[:])

            # q/k/v projections (weights-stationary, 8 accum steps)
            for wname, bname, dst in (("wq", "bq", qT), ("wk", "bk", kT),
                                      ("wv", None, vT)):
                psq = ps.tile([FPC, 512], F32, tag="sc",
                              name=f"ps{wname}{th}")
                for d in range(D // P):
                    nc.tensor.matmul(psq[:], w_ts[wname][:, d, :], xts[d][:],
                                     start=(d == 0), stop=(d == D // P - 1))
                if bname is not None:
                    nc.vector.tensor_scalar(
                        out=dst[:, sl], in0=psq[:], scalar1=b_ts[bname][:],
                        scalar2=None, op0=ALU.add)
                else:
                    nc.vector.tensor_copy(dst[:, sl], psq[:])

            if th % 4 == 3:          # batch b fully projected
                emit_swaps(b)
                emit_vaug(b)

        # ---------------- phase B: attention + output projection ----------
        blocks = [(0, 0), (1, 0), (0, 1), (1, 1)]
        pending_po = []              # (b, tt) out-proj chunks to interleave

        def emit_po(b, tt):
            po = ps.tile([P, 1024], F32, tag="sc", name=f"po{b}{tt}")
            lo = OT[b][:, tt * P:(tt + 1) * P]
            nc.tensor.matmul(po[:, 0:512], lo, wo_t[:, 0:512],
                             start=True, stop=True)
            nc.tensor.matmul(po[:, 512:1024], lo, wo_t[:, 512:1024],
                             start=True, stop=True)
            ob = ob_pool.tile([P, 1024], FP16, tag="ob", name=f"ob{b}{tt}")
            nc.vector.tensor_copy(ob[:], po[:])
            nc.sync.dma_start(out[b * T + tt * P: b * T + (tt + 1) * P, :],
                              ob[:])

        for h, b in blocks:
            hsl = slice(h * HD, (h + 1) * HD)
            va_t = va[(h, b)]
            gbc_t = gbc[(h, b)]
            kA = (kT if h == 0 else ksw)
            kB = (ksw if h == 0 else kT)
            qA = (qT if h == 0 else qsw)
            qB = (qsw if h == 0 else qT)
            avs = [ps.tile([HD + 1, 512], F32, tag="av", bufs=4,
                           name=f"avs{h}{b}{qq}") for qq in range(4)]
            def emit_av(kc, exs):
                vak = va_t[:, kc * (HD + 1):(kc + 1) * (HD + 1)]
                for pi in range(2):
                    nc.tensor.matmul(avs[2 * pi][:], vak, exs[pi][:, 0:512],
                                     start=(kc == 0), stop=(kc == NKC - 1))
                    nc.tensor.matmul(avs[2 * pi + 1][:], vak,
                                     exs[pi][:, 512:1024],
                                     start=(kc == 0), stop=(kc == NKC - 1))

            prev_av = None
            for kc in range(NKC):
                if pending_po:
                    emit_po(*pending_po.pop(0))
                pbt_t = pb_pool.tile([P, T], BF16, tag="pb",
                                     name=f"pbt{h}{b}{kc}")
                nc.sync.dma_start(pbt_t[:], pbt[h, kc * P:(kc + 1) * P, :])
                pbg = pbg_pool.tile([P, T], BF16, tag="pbg",
                                    name=f"pbg{h}{b}{kc}")
                nc.vector.tensor_tensor(out=pbg[:], in0=pbt_t[:],
                                        in1=gbc_t[:], op=ALU.mult)
                ksl = slice(b * T + kc * P, b * T + (kc + 1) * P)
                sc2s = []
                for pi in range(2):
                    q0 = b * T + pi * 1024
                    sc2 = ps.tile([P, 1024], F32, tag="sc",
                                  name=f"sc{h}{b}{kc}{pi}")
                    nc.tensor.matmul(sc2[:, 0:512], kA[0:HD, ksl],
                                     qA[0:HD, q0:q0 + 512],
                                     start=True, stop=False)
                    nc.tensor.matmul(sc2[:, 512:1024], kB[HD:FPC, ksl],
                                     qB[HD:FPC, q0 + 512:q0 + 1024],
                                     start=True, stop=False)
                    sc2s.append(sc2)
                for pi in range(2):
                    nc.tensor.matmul(sc2s[pi][:, 0:512], idb_t[:],
                                     pbg[:, pi * 1024:pi * 1024 + 512],
                                     start=False, stop=True)
                    nc.tensor.matmul(sc2s[pi][:, 512:1024], idb_t[:],
                                     pbg[:, pi * 1024 + 512:(pi + 1) * 1024],
                                     start=False, stop=True)
                exs = []
                for pi in range(2):
                    ex = ex_pool.tile([P, 1024], BF16, tag="ex",
                                      name=f"ex{h}{b}{kc}{pi}")
                    nc.scalar.activation(ex[:], sc2s[pi][:], AF.Exp)
                    exs.append(ex)
                if prev_av is not None:
                    emit_av(*prev_av)
                prev_av = (kc, exs)
            emit_av(*prev_av)
            # normalization: reciprocal of the free denominators (row HD),
            # PE-broadcast, multiply during avs evacuation into OT
            for qq in range(4):
                dsc = dsc_pool.tile([HD + 1, 512], BF16, tag="dsc",
                                    name=f"dsc{h}{b}{qq}")
                nc.scalar.copy(dsc[HD:HD + 1, :], avs[qq][HD:HD + 1, :])
                rbp = ps.tile([HD, 512], F32, tag="sc",
                              name=f"rbp{h}{b}{qq}")
                nc.tensor.matmul(rbp[:], ones1[HD:HD + 1, 0:HD],
                                 dsc[HD:HD + 1, :], start=True, stop=True)
                rbr = rbr_pool.tile([HD, 512], F32, tag="rbr",
                                    name=f"rbr{h}{b}{qq}")
                nc.vector.reciprocal_approx_fast(rbr[:], rbp[:])
                nc.vector.tensor_tensor(
                    out=OT[b][hsl, qq * 512:(qq + 1) * 512],
                    in0=avs[qq][0:HD, :], in1=rbr[:], op=ALU.mult)
            if h == 1:
                pending_po.extend((b, tt) for tt in range(T // P))

        while pending_po:
            emit_po(*pending_po.pop(0))

    nc.compile()
    return nc


_PROGRAM = None


def _get_program():
    global _PROGRAM
    if _PROGRAM is None:
        _PROGRAM = _build_program()
    return _PROGRAM


def kernel(x, position_bias, Wq, bq, Wk, bk, Wv, bv, Wo, bo, Wg, bg,
           gru_const):
    global LAST_RESULT
    x = np.asarray(x, dtype=np.float32)
    position_bias = np.asarray(position_bias, dtype=np.float32)
    Wq = np.asarray(Wq, dtype=np.float32)
    Wk = np.asarray(Wk, dtype=np.float32)
    Wv = np.asarray(Wv, dtype=np.float32)
    Wo = np.asarray(Wo, dtype=np.float32)
    bq = np.asarray(bq, dtype=np.float32)
    bk = np.asarray(bk, dtype=np.float32)
    bv = np.asarray(bv, dtype=np.float32)
    bo = np.asarray(bo, dtype=np.float32)
    Wg = np.asarray(Wg, dtype=np.float32)
    bg = np.asarray(bg, dtype=np.float32)
    gru_const = np.asarray(gru_const, dtype=np.float32)

    scale = np.float32(1.0 / np.sqrt(np.float32(HD)))

    xT_np = np.ascontiguousarray(x.reshape(BT, D).T)           # [D, BT]
    idb_np = np.eye(P).astype(ml_dtypes.bfloat16)
    # the reshape-(2,4)-sum of the 8 gate features is linear -> fold into
    # the weights:  Wg2[g] = sum of Wg rows [4g, 4g+4)
    Wg2 = Wg.reshape(2, 4, HD).sum(1)                          # [2, HD]
    bg2v = bg.reshape(2, 4).sum(1)                             # [2]

    in_maps = []
    for c in range(NCORES):
        fsl = slice(c * FPC, (c + 1) * FPC)
        wg2_np = np.zeros((P, 97), dtype=np.float32)
        bg2_np = np.zeros((97,), dtype=np.float32)
        # rows 0/32 = gate-a for head0/head1; rows 64/96 = gate-b
        wg2_np[0:HD, 0] = Wg2[0]
        wg2_np[HD:P, 32] = Wg2[0]
        wg2_np[0:HD, 64] = Wg2[1]
        wg2_np[HD:P, 96] = Wg2[1]
        bg2_np[[0, 32]] = bg2v[0]
        bg2_np[[64, 96]] = bg2v[1]
        gc2_np = np.zeros((97,), dtype=np.float32)
        gc2_np[64] = gru_const[0, c * HPC, 0, 0]
        gc2_np[96] = gru_const[0, c * HPC + 1, 0, 0]
        in_maps.append({
            "xT": xT_np.astype(ml_dtypes.bfloat16),
            "xg": np.ascontiguousarray(xT_np[fsl, :]).astype(ml_dtypes.bfloat16),
            "wq": (np.ascontiguousarray(Wq.T[:, fsl]) * scale).astype(ml_dtypes.bfloat16),
            "wk": np.ascontiguousarray(Wk.T[:, fsl]).astype(ml_dtypes.bfloat16),
            "wv": np.ascontiguousarray(Wv.T[:, fsl]).astype(ml_dtypes.bfloat16),
            "bq": np.ascontiguousarray(bq[fsl]) * scale,
            "bk": np.ascontiguousarray(bk[fsl]),
            "wo": np.ascontiguousarray(Wo[:, fsl].T).astype(ml_dtypes.bfloat16),
            "pbt": np.ascontiguousarray(
                position_bias[c * HPC:(c + 1) * HPC].transpose(0, 2, 1)
            ).astype(ml_dtypes.bfloat16),
            "wg2": wg2_np.astype(ml_dtypes.bfloat16),
            "bg2": bg2_np,
            "gc2": gc2_np,
            "idb": idb_np,
        })

    nc = _get_program()
    res = run_bass_kernel_spmd(nc, in_maps, core_ids=list(range(NCORES)),
                               trace=TRACE)
    LAST_RESULT = res
    acc = res.results[0]["out"].astype(np.float32).copy()
    for c in range(1, NCORES):
        acc += res.results[c]["out"].astype(np.float32)
    # v-bias folds exactly through the projection (attn rows sum to 1)
    acc += bo[None, :] + (bv @ Wo.T)[None, :]
    return acc.reshape(B, T, D)


# revision 13
# speedup vs baseline: 1.1918x; 1.1086x over previous
"""Gated-relative-position-bias multi-head attention, 8-way tensor-parallel
over heads on Trainium2 (Bass/Tile).  v2 — PE-stall-free redesign.

Shapes: x (2, 2048, 1024), 16 heads x 64 head-dim, position_bias
(16, 2048, 2048), per-query sigmoid gates computed from x.

Sharding: core c owns heads (2c, 2c+1) = feature slice [128c, 128c+128).
Each core computes q/k/v for its heads, the gated-bias attention, and a
partial output projection (O_g @ Wo_g.T) written in fp16.  The host sums
the 8 partials and adds bo (+ the exact fold of bv through Wo: attention
rows sum to 1, so v-bias contributes bv @ Wo.T to every output row).

Key structure (all engines balanced, PE never waits):
  - scores are computed TRANSPOSED, sT[k, q] = kT.T @ qT (K=hd=64), with
    TWO query-halves run CONCURRENTLY via PE row tiling: the second
    matmul uses partition-swapped copies of k/q (kswap/qswap, built by
    SBUF->SBUF DMA) so it lands on PE rows 64-127 while the first uses
    rows 0-63.  This halves score-matmul wall time.
  - the gated position bias is added into the score PSUM by the PE via
    an identity matmul (psum += I.T @ pbg); pbg = pbT * gate_bcast is
    formed on the DVE in bf16 (2x mode).  gate_bcast is built by a K=1
    ones-matmul on the PE (no gpsimd anywhere in this kernel).
  - softmax needs no max-subtraction (scores are O(+-1) for this model
    family); denominators come free as an all-ones column of v_aug
    (row 64 of the AV matmul PSUM output).
  - exp runs on ACT as one [128,1024] pass per query-pair straight out
    of the 2-bank score PSUM tile.
  - normalization happens during the avs PSUM evacuation (DVE multiply
    by a PE-broadcast reciprocal); the output projection runs on the
    normalized OT and is evacuated by DVE into fp16.
"""

import sys

sys.path.insert(0, "/opt/trn_rl_repo")

import ml_dtypes
import numpy as np

import concourse.mybir as mybir
import concourse.tile as tile
from concourse import bacc
from concourse.bass_utils import run_bass_kernel_spmd

F32 = mybir.dt.float32
BF16 = mybir.dt.bfloat16
FP16 = mybir.dt.float16
AF = mybir.ActivationFunctionType
ALU = mybir.AluOpType

B, T, D, H, HD = 2, 2048, 1024, 16, 64
NCORES = 8
HPC = H // NCORES          # heads per core = 2
FPC = HPC * HD             # features per core = 128
BT = B * T                 # 4096
P = 128
NKC = T // P               # key chunks per (h, b) = 16
NTH = BT // 512            # 512-col projection tiles = 8

# test.py hooks
TRACE = False
LAST_RESULT = None


def _build_program():
    nc = bacc.Bacc("TRN2", target_bir_lowering=False, debug=False,
                   num_devices=NCORES)

    xT = nc.dram_tensor("xT", [D, BT], BF16, kind="ExternalInput")
    xg = nc.dram_tensor("xg", [P, BT], BF16, kind="ExternalInput")
    wq = nc.dram_tensor("wq", [D, FPC], BF16, kind="ExternalInput")
    wk = nc.dram_tensor("wk", [D, FPC], BF16, kind="ExternalInput")
    wv = nc.dram_tensor("wv", [D, FPC], BF16, kind="ExternalInput")
    bq = nc.dram_tensor("bq", [FPC], F32, kind="ExternalInput")
    bk = nc.dram_tensor("bk", [FPC], F32, kind="ExternalInput")
    wo = nc.dram_tensor("wo", [FPC, D], BF16, kind="ExternalInput")
    pbt = nc.dram_tensor("pbt", [HPC, T, T], BF16, kind="ExternalInput")
    wg2 = nc.dram_tensor("wg2", [P, 97], BF16, kind="ExternalInput")
    bg2 = nc.dram_tensor("bg2", [97], F32, kind="ExternalInput")
    gc2 = nc.dram_tensor("gc2", [97], F32, kind="ExternalInput")
    idb = nc.dram_tensor("idb", [P, P], BF16, kind="ExternalInput")
    out = nc.dram_tensor("out", [BT, D], FP16, kind="ExternalOutput")

    with tile.TileContext(nc) as tc, \
         tc.tile_pool(name="const", bufs=1) as const, \
         tc.tile_pool(name="big", bufs=1) as big, \
         tc.tile_pool(name="xt", bufs=2) as xt_pool, \
         tc.tile_pool(name="gtmp", bufs=2) as gtmp_pool, \
         tc.tile_pool(name="pb", bufs=3) as pb_pool, \
         tc.tile_pool(name="pbgp", bufs=2) as pbg_pool, \
         tc.tile_pool(name="exp", bufs=4) as ex_pool, \
         tc.tile_pool(name="dscp", bufs=4) as dsc_pool, \
         tc.tile_pool(name="rbrp", bufs=2) as rbr_pool, \
         tc.tile_pool(name="obp", bufs=3) as ob_pool, \
         tc.tile_pool(name="ps", bufs=2, space="PSUM") as ps:
        # ---------------- constants (latency-ordered DMAs) ----------------
        ones1 = const.tile([P, P], BF16, tag="ones")
        nc.vector.memset(ones1[:], 1.0)
        wg2_t = const.tile([P, 97], BF16, tag="wg2")
        nc.sync.dma_start(wg2_t[:], wg2[:])
        bg2_t = const.tile([97, 1], F32, tag="bg2")
        nc.sync.dma_start(bg2_t[:], bg2.rearrange("(p o) -> p o", o=1))
        gc_t = const.tile([97, 1], F32, tag="gc")
        nc.sync.dma_start(gc_t[:], gc2.rearrange("(p o) -> p o", o=1))
        b_ts = {}
        for name, dram in (("bq", bq), ("bk", bk)):
            b_t = const.tile([FPC, 1], F32, tag=name, name=name + "b")
            nc.sync.dma_start(b_t[:], dram.rearrange("(p o) -> p o", o=1))
            b_ts[name] = b_t
        w_ts = {}
        for name, dram in (("wq", wq), ("wk", wk), ("wv", wv)):
            w_t = const.tile([P, D // P, FPC], BF16, tag=name, name=name + "w")
            nc.sync.dma_start(w_t[:], dram.rearrange("(c p) f -> p c f", p=P))
            w_ts[name] = w_t
        idb_t = const.tile([P, P], BF16, tag="idb")
        nc.sync.dma_start(idb_t[:], idb[:])
        wo_t = const.tile([FPC, D], BF16, tag="wo")
        nc.sync.dma_start(wo_t[:], wo[:])

        qT = big.tile([FPC, BT], BF16, tag="qT")
        kT = big.tile([FPC, BT], BF16, tag="kT")
        qsw = big.tile([FPC, BT], BF16, tag="qsw")
        ksw = big.tile([FPC, BT], BF16, tag="ksw")
        vT = big.tile([FPC, BT], BF16, tag="vT")
        G = big.tile([97, BT], BF16, tag="G")
        G2h = [big.tile([1, BT], BF16, tag=f"G2h{h}", name=f"G2h{h}")
               for h in range(HPC)]
        gbc = {(h, b): big.tile([P, T], BF16, tag=f"gbc{h}{b}",
                                name=f"gbc{h}{b}")
               for h in range(HPC) for b in range(B)}
        va = {(h, b): big.tile([P, NKC * (HD + 1)], BF16,
                               tag=f"va{h}{b}", name=f"va{h}{b}")
              for h in range(HPC) for b in range(B)}
        OT = [big.tile([FPC, T], BF16, tag=f"OT{b}", name=f"OT{b}")
              for b in range(B)]

        # ones-columns of v_aug (data columns are overwritten by the
        # transpose evacuations below)
        for h in range(HPC):
            for b in range(B):
                vac = va[(h, b)][:, :].rearrange("p (k c) -> p k c", c=HD + 1)
                nc.vector.memset(vac[:, :, HD:HD + 1], 1.0)

        # ---------------- phase A: q/k/v projections + gate ----------------
        def emit_vaug(b):
            # vT[:, b] -> per-head transposed v chunks in va (PE row-paired
            # transposes, staged 8-at-a-time in a half PSUM bank)
            for half in range(2):
                stages = []
                for h in range(HPC):
                    hsl = slice(h * HD, (h + 1) * HD)
                    stage = ps.tile([P, 512], BF16, tag="av", bufs=4,
                                    name=f"vstg{b}{half}{h}")
                    for j in range(8):
                        kc = half * 8 + j
                        nc.tensor.transpose(
                            stage[:, j * HD:(j + 1) * HD],
                            vT[hsl, b * T + kc * P: b * T + (kc + 1) * P],
                            idb_t[hsl, hsl])
                    stages.append(stage)
                for h in range(HPC):
                    dst = va[(h, b)][:, half * 8 * (HD + 1):
                                     (half * 8 + 8) * (HD + 1)]
                    dst = dst.rearrange("p (k c) -> p k c", c=HD + 1)
                    src = stages[h][:, :].rearrange("p (k c) -> p k c", c=HD)
                    nc.scalar.copy(dst[:, :, 0:HD], src[:])

        def emit_swaps(b):
            bsl = slice(b * T, (b + 1) * T)
            for src, dstt in ((kT, ksw), (qT, qsw)):
                nc.sync.dma_start(dstt[0:HD, bsl], src[HD:FPC, bsl])
                nc.sync.dma_start(dstt[HD:FPC, bsl], src[0:HD, bsl])

        for th in range(NTH):
            b = th // (NTH // B)
            sl = slice(th * 512, (th + 1) * 512)
            qsl = slice((th % 4) * 512, (th % 4) * 512 + 512)
            xt_t = xt_pool.tile([P, D // P, 512], BF16, tag="xt", bufs=3,
                                name=f"xt_{th}")
            nc.sync.dma_start(
                xt_t[:], xT.rearrange("(c p) t -> p c t", p=P)[:, :, sl])
            xts = [xt_t[:, d, :] for d in range(D // P)]
            xg_t = xt_pool.tile([P, 512], BF16, tag="xg", bufs=3,
                                name=f"xg_{th}")
            nc.sync.dma_start(xg_t[:], xg[:, sl])

            # gate: projection, sigmoid, per-head combine, PE broadcast
            psg = ps.tile([97, 512], F32, tag="sc", name=f"psg{th}")
            nc.tensor.matmul(psg[:], wg2_t[:], xg_t[:], start=True, stop=True)
            nc.scalar.activation(G[:, sl], psg[:], AF.Sigmoid, bias=bg2_t[:])
            gt1 = gtmp_pool.tile([97, 512], BF16, tag="gt1", name=f"gt1_{th}")
            for h in range(HPC):
                r = 32 * h
                nc.vector.tensor_scalar(
                    out=gt1[r:r + 1, :], in0=G[64 + r:65 + r, sl],
                    scalar1=gc_t[64 + r:65 + r, :], scalar2=-1.0,
                    op0=ALU.mult, op1=ALU.add)
                nc.vector.tensor_tensor(
                    out=G2h[h][0:1, sl], in0=G[r:r + 1, sl],
                    in1=gt1[r:r + 1, :], op=ALU.mult)
                nc.vector.tensor_scalar(
                    out=G2h[h][0:1, sl], in0=G2h[h][0:1, sl],
                    scalar1=2.0, scalar2=None, op0=ALU.add)
                psb = ps.tile([P, 512], F32, tag="sc", name=f"psb{th}{h}")
                nc.tensor.matmul(psb[:], ones1[0:1, :], G2h[h][0:1, sl],
                                 start=True, stop=True)
                if h == 0:
                    nc.scalar.copy(gbc[(h, b)][:, qsl], psb[:])
                else:
                    nc.vector.tensor_copy(gbc[(h, b)][:, qsl], psb[:])

            # q/k/v projections (weights-stationary, 8 accum steps)
            for wname, bname, dst in (("wq", "bq", qT), ("wk", "bk", kT),
                                      ("wv", None, vT)):
                psq = ps.tile([FPC, 512], F32, tag="av", bufs=4,
                              name=f"ps{wname}{th}")
                for d in range(D // P):
                    nc.tensor.matmul(psq[:], w_ts[wname][:, d, :], xts[d][:],
                                     start=(d == 0), stop=(d == D // P - 1))
                if bname is not None:
                    nc.scalar.activation(dst[:, sl], psq[:], AF.Identity,
                                         bias=b_ts[bname][:])
                else:
                    nc.vector.tensor_copy(dst[:, sl], psq[:])

            if th % 4 == 3:          # batch b fully projected
                emit_swaps(b)
                emit_vaug(b)

        # ---------------- phase B: attention + output projection ----------
        blocks = [(0, 0), (1, 0), (0, 1), (1, 1)]
        pending_po = []              # (b, tt) out-proj chunks to interleave

        def emit_po(b, tt, on_act=False):
            po = ps.tile([P, 1024], F32, tag="sc", name=f"po{b}{tt}")
            lo = OT[b][:, tt * P:(tt + 1) * P]
            nc.tensor.matmul(po[:, 0:512], lo, wo_t[:, 0:512],
                             start=True, stop=True)
            nc.tensor.matmul(po[:, 512:1024], lo, wo_t[:, 512:1024],
                             start=True, stop=True)
            ob = ob_pool.tile([P, 1024], FP16, tag="ob", name=f"ob{b}{tt}")
            if on_act:
                nc.scalar.copy(ob[:], po[:])
            else:
                nc.vector.tensor_copy(ob[:], po[:])
            nc.sync.dma_start(out[b * T + tt * P: b * T + (tt + 1) * P, :],
                              ob[:])

        for h, b in blocks:
            hsl = slice(h * HD, (h + 1) * HD)
            va_t = va[(h, b)]
            gbc_t = gbc[(h, b)]
            kA = (kT if h == 0 else ksw)
            kB = (ksw if h == 0 else kT)
            qA = (qT if h == 0 else qsw)
            qB = (qsw if h == 0 else qT)
            avs = [ps.tile([HD + 1, 512], F32, tag="av", bufs=4,
                           name=f"avs{h}{b}{qq}") for qq in range(4)]
            def emit_av(kc, exs):
                vak = va_t[:, kc * (HD + 1):(kc + 1) * (HD + 1)]
                for pi in range(2):
                    nc.tensor.matmul(avs[2 * pi][:], vak, exs[pi][:, 0:512],
                                     start=(kc == 0), stop=(kc == NKC - 1))
                    nc.tensor.matmul(avs[2 * pi + 1][:], vak,
                                     exs[pi][:, 512:1024],
                                     start=(kc == 0), stop=(kc == NKC - 1))

            prev_av = None
            for kc in range(NKC):
                if pending_po:
                    emit_po(*pending_po.pop(0))
                pbt_t = pb_pool.tile([P, T], BF16, tag="pb",
                                     name=f"pbt{h}{b}{kc}")
                nc.sync.dma_start(pbt_t[:], pbt[h, kc * P:(kc + 1) * P, :])
                pbg = pbg_pool.tile([P, T], BF16, tag="pbg",
                                    name=f"pbg{h}{b}{kc}")
                nc.vector.tensor_tensor(out=pbg[:], in0=pbt_t[:],
                                        in1=gbc_t[:], op=ALU.mult)
                ksl = slice(b * T + kc * P, b * T + (kc + 1) * P)
                sc2s = []
                for pi in range(2):
                    q0 = b * T + pi * 1024
                    sc2 = ps.tile([P, 1024], F32, tag="sc",
                                  name=f"sc{h}{b}{kc}{pi}")
                    nc.tensor.matmul(sc2[:, 0:512], kA[0:HD, ksl],
                                     qA[0:HD, q0:q0 + 512],
                                     start=True, stop=False)
                    nc.tensor.matmul(sc2[:, 512:1024], kB[HD:FPC, ksl],
                                     qB[HD:FPC, q0 + 512:q0 + 1024],
                                     start=True, stop=False)
                    sc2s.append(sc2)
                for pi in range(2):
                    nc.tensor.matmul(sc2s[pi][:, 0:512], idb_t[:],
                                     pbg[:, pi * 1024:pi * 1024 + 512],
                                     start=False, stop=True)
                    nc.tensor.matmul(sc2s[pi][:, 512:1024], idb_t[:],
                                     pbg[:, pi * 1024 + 512:(pi + 1) * 1024],
                                     start=False, stop=True)
                exs = []
                for pi in range(2):
                    ex = ex_pool.tile([P, 1024], BF16, tag="ex",
                                      name=f"ex{h}{b}{kc}{pi}")
                    nc.scalar.activation(ex[:], sc2s[pi][:], AF.Exp)
                    exs.append(ex)
                if prev_av is not None:
                    emit_av(*prev_av)
                prev_av = (kc, exs)
            emit_av(*prev_av)
            # normalization: reciprocal of the free denominators (row HD),
            # PE-broadcast, multiply during avs evacuation into OT
            for qq in range(4):
                dsc = dsc_pool.tile([HD + 1, 512], BF16, tag="dsc",
                                    name=f"dsc{h}{b}{qq}")
                nc.scalar.copy(dsc[HD:HD + 1, :], avs[qq][HD:HD + 1, :])
                rbp = ps.tile([HD, 512], F32, tag="sc",
                              name=f"rbp{h}{b}{qq}")
                nc.tensor.matmul(rbp[:], ones1[HD:HD + 1, 0:HD],
                                 dsc[HD:HD + 1, :], start=True, stop=True)
                rbr = rbr_pool.tile([HD, 512], F32, tag="rbr",
                                    name=f"rbr{h}{b}{qq}")
                nc.vector.reciprocal_approx_fast(rbr[:], rbp[:])
                nc.vector.tensor_tensor(
                    out=OT[b][hsl, qq * 512:(qq + 1) * 512],
                    in0=avs[qq][0:HD, :], in1=rbr[:], op=ALU.mult)
                if h == 1 and b == B - 1:
                    # last block: drain the out-projection per query-quarter,
                    # evacuations alternating DVE/ACT (both idle by now)
                    for tt in range(4 * qq, 4 * qq + 4):
                        emit_po(b, tt, on_act=(tt % 2 == 1))
            if h == 1 and b < B - 1:
                pending_po.extend((b, tt) for tt in range(T // P))

    nc.compile()
    return nc


_PROGRAM = None


def _get_program():
    global _PROGRAM
    if _PROGRAM is None:
        _PROGRAM = _build_program()
    return _PROGRAM


def kernel(x, position_bias, Wq, bq, Wk, bk, Wv, bv, Wo, bo, Wg, bg,
           gru_const):
    global LAST_RESULT
    x = np.asarray(x, dtype=np.float32)
    position_bias = np.asarray(position_bias, dtype=np.float32)
    Wq = np.asarray(Wq, dtype=np.float32)
    Wk = np.asarray(Wk, dtype=np.float32)
    Wv = np.asarray(Wv, dtype=np.float32)
    Wo = np.asarray(Wo, dtype=np.float32)
    bq = np.asarray(bq, dtype=np.float32)
    bk = np.asarray(bk, dtype=np.float32)
    bv = np.asarray(bv, dtype=np.float32)
    bo = np.asarray(bo, dtype=np.float32)
    Wg = np.asarray(Wg, dtype=np.float32)
    bg = np.asarray(bg, dtype=np.float32)
    gru_const = np.asarray(gru_const, dtype=np.float32)

    scale = np.float32(1.0 / np.sqrt(np.float32(HD)))

    xT_np = np.ascontiguousarray(x.reshape(BT, D).T)           # [D, BT]
    idb_np = np.eye(P).astype(ml_dtypes.bfloat16)
    # the reshape-(2,4)-sum of the 8 gate features is linear -> fold into
    # the weights:  Wg2[g] = sum of Wg rows [4g, 4g+4)
    Wg2 = Wg.reshape(2, 4, HD).sum(1)                          # [2, HD]
    bg2v = bg.reshape(2, 4).sum(1)                             # [2]

    in_maps = []
    for c in range(NCORES):
        fsl = slice(c * FPC, (c + 1) * FPC)
        wg2_np = np.zeros((P, 97), dtype=np.float32)
        bg2_np = np.zeros((97,), dtype=np.float32)
        # rows 0/32 = gate-a for head0/head1; rows 64/96 = gate-b
        wg2_np[0:HD, 0] = Wg2[0]
        wg2_np[HD:P, 32] = Wg2[0]
        wg2_np[0:HD, 64] = Wg2[1]
        wg2_np[HD:P, 96] = Wg2[1]
        bg2_np[[0, 32]] = bg2v[0]
        bg2_np[[64, 96]] = bg2v[1]
        gc2_np = np.zeros((97,), dtype=np.float32)
        gc2_np[64] = gru_const[0, c * HPC, 0, 0]
        gc2_np[96] = gru_const[0, c * HPC + 1, 0, 0]
        in_maps.append({
            "xT": xT_np.astype(ml_dtypes.bfloat16),
            "xg": np.ascontiguousarray(xT_np[fsl, :]).astype(ml_dtypes.bfloat16),
            "wq": (np.ascontiguousarray(Wq.T[:, fsl]) * scale).astype(ml_dtypes.bfloat16),
            "wk": np.ascontiguousarray(Wk.T[:, fsl]).astype(ml_dtypes.bfloat16),
            "wv": np.ascontiguousarray(Wv.T[:, fsl]).astype(ml_dtypes.bfloat16),
            "bq": np.ascontiguousarray(bq[fsl]) * scale,
            "bk": np.ascontiguousarray(bk[fsl]),
            "wo": np.ascontiguousarray(Wo[:, fsl].T).astype(ml_dtypes.bfloat16),
            "pbt": np.ascontiguousarray(
                position_bias[c * HPC:(c + 1) * HPC].transpose(0, 2, 1)
            ).astype(ml_dtypes.bfloat16),
            "wg2": wg2_np.astype(ml_dtypes.bfloat16),
            "bg2": bg2_np,
            "gc2": gc2_np,
            "idb": idb_np,
        })

    nc = _get_program()
    res = run_bass_kernel_spmd(nc, in_maps, core_ids=list(range(NCORES)),
                               trace=TRACE)
    LAST_RESULT = res
    acc = res.results[0]["out"].astype(np.float32).copy()
    for c in range(1, NCORES):
        acc += res.results[c]["out"].astype(np.float32)
    # v-bias folds exactly through the projection (attn rows sum to 1)
    acc += bo[None, :] + (bv @ Wo.T)[None, :]
    return acc.reshape(B, T, D)
